# revision 5
# baseline (speedup 1.0000x reference)
"""Trainium2 Bass kernel for a 2-layer directed GCN (PyG GCNConv semantics).

Strategy (8-core SPMD, 1D node sharding):
  - Nodes sharded across 8 cores (12500 each, padded to 12544 = 98*128).
  - Edges partitioned by destination core; per core grouped by destination
    tile (128 nodes), sub-grouped by source chunk (int16 gather reach), and
    padded to whole 128-edge tiles with a cross-core-uniform tile count so a
    single SPMD program serves all cores.
  - Per-edge coefficients nrm = dinv[row]*ew*dinv[col] and self-loop weights
    dinv^2 are host-precomputed (graph-structure preprocessing, cacheable).
  - Linearity trick: aggregate raw features first, apply W afterwards:
        out[c] = (sum_e nrm_e * x[row_e] + dinv_c^2 * x_c) @ W + b
    so layer 1 needs no feature AllGather of x@W; one AllGather replicates
    the x slabs and one AllGather of relu(out1) feeds layer 2.
  - Per 128-edge tile on device: batched gpsimd dma_gather of source rows,
    scaled one-hot S[e,d] = (iota==col)*nrm via one fused DVE tensor_scalar,
    TensorE matmul S^T @ msgs accumulating the destination tile in PSUM.
    Supertiles of 6 destination tiles keep 6 PSUM aggregation banks live
    across the 4 source-chunk passes (+1 transpose +1 output bank = 8).
  - Per destination tile tail: self term (DVE scalar_tensor_tensor), PE
    transpose, x W (PE), bias via K=1 rank-1 matmul, relu/copy evict (ACT).
"""

from contextlib import ExitStack

import numpy as np

import concourse.bacc as bacc
import concourse.bass as bass
import concourse.mybir as mybir
import concourse.tile as tile
from concourse.bass_utils import run_bass_kernel_spmd
from concourse.library_config import mlp

N_NODES = 100000
D = 128
N_CORES = 8
NPC = N_NODES // N_CORES          # 12500 nodes per core
TPC = (NPC + 127) // 128          # 98 destination tiles per core
PAD_NPC = TPC * 128               # 12544 padded nodes per core
N_PAD = N_CORES * PAD_NPC         # 100352 padded table rows
NCHUNK = 4
CHUNK_ROWS = N_PAD // NCHUNK      # 25088 (< 32768, fits int16 indices)
SUPER = 6                         # dest tiles per supertile (PSUM: 6+1+1)

F32 = mybir.dt.float32
I16 = mybir.dt.int16


def _build_nc(t_counts):
    """t_counts[chunk][dtile] = edge tiles (uniform across cores)."""
    NSUP = (TPC + SUPER - 1) // SUPER
    NT = int(sum(t_counts[c][d] for c in range(NCHUNK) for d in range(TPC)))
    nc = bacc.Bacc("TRN2", target_bir_lowering=False)

    x_slab = nc.dram_tensor("x_slab", [PAD_NPC, D], F32, kind="ExternalInput")
    gix = nc.dram_tensor("gix", [128, NT * 8], I16, kind="ExternalInput")
    colw = nc.dram_tensor("colw", [128, NT], F32, kind="ExternalInput")
    nrmw = nc.dram_tensor("nrmw", [128, NT], F32, kind="ExternalInput")
    selfw = nc.dram_tensor("selfw", [128, TPC], F32, kind="ExternalInput")
    w1 = nc.dram_tensor("w1", [D, D], F32, kind="ExternalInput")
    b1 = nc.dram_tensor("b1", [1, D], F32, kind="ExternalInput")
    w2 = nc.dram_tensor("w2", [D, D], F32, kind="ExternalInput")
    b2 = nc.dram_tensor("b2", [1, D], F32, kind="ExternalInput")
    ident = nc.dram_tensor("ident", [128, 128], F32, kind="ExternalInput")
    iota = nc.dram_tensor("iota", [128, 128], F32, kind="ExternalInput")
    ones = nc.dram_tensor("ones", [1, D], F32, kind="ExternalInput")
    out_slab = nc.dram_tensor("out_slab", [NPC, D], F32, kind="ExternalOutput")

    # per-(super, chunk) contiguous tile segments
    seg_tiles = [[sum(t_counts[c][d]
                      for d in range(s * SUPER, min((s + 1) * SUPER, TPC)))
                  for c in range(NCHUNK)] for s in range(NSUP)]
    max_seg = max(max(row) for row in seg_tiles) or 1

    with tile.TileContext(nc) as tc:
        nc.gpsimd.load_library(mlp)
        with (
            tc.tile_pool(name="const", bufs=1) as constp,
            tc.tile_pool(name="gbuf", bufs=3) as gbufp,
            tc.tile_pool(name="sbld", bufs=4) as sbldp,
            tc.tile_pool(name="tailp", bufs=3) as tailp,
            tc.tile_pool(name="psag", bufs=SUPER, space="PSUM") as psagp,
            tc.tile_pool(name="pst", bufs=1, space="PSUM") as pstp,
            tc.tile_pool(name="pso", bufs=1, space="PSUM") as psop,
            tc.tile_pool(name="dram", bufs=1, space="DRAM") as dramp,
        ):
            gix_s = constp.tile([128, NT * 8], I16, tag="gix")
            colw_s = constp.tile([128, NT], F32, tag="colw")
            nrmw_s = constp.tile([128, NT], F32, tag="nrmw")
            selfw_s = constp.tile([128, TPC], F32, tag="selfw")
            w1_s = constp.tile([D, D], F32, tag="w1")
            b1_s = constp.tile([1, D], F32, tag="b1")
            w2_s = constp.tile([D, D], F32, tag="w2")
            b2_s = constp.tile([1, D], F32, tag="b2")
            id_s = constp.tile([128, 128], F32, tag="ident")
            iota_s = constp.tile([128, 128], F32, tag="iota")
            ones_s = constp.tile([1, D], F32, tag="ones")
            for dst, srct in ((gix_s, gix), (colw_s, colw), (nrmw_s, nrmw),
                              (selfw_s, selfw), (w1_s, w1), (b1_s, b1),
                              (w2_s, w2), (b2_s, b2), (id_s, ident),
                              (iota_s, iota), (ones_s, ones)):
                nc.sync.dma_start(dst[:], srct[:])

            x_cp = dramp.tile([PAD_NPC, D], F32, tag="x_cp")
            x_full = dramp.tile([N_PAD, D], F32, tag="x_full")
            h_local = dramp.tile([PAD_NPC, D], F32, tag="h_local")
            h_full = dramp.tile([N_PAD, D], F32, tag="h_full")
            nc.gpsimd.dma_start(x_cp[:], x_slab[:])
            nc.gpsimd.collective_compute(
                "AllGather", mybir.AluOpType.bypass,
                replica_groups=[list(range(N_CORES))],
                ins=[x_cp.opt()], outs=[x_full.opt()])

            def layer(table, self_src, w_s, b_s, relu, store):
                t0 = 0
                for s in range(NSUP):
                    dlist = list(range(s * SUPER, min((s + 1) * SUPER, TPC)))
                    total_d = {d: sum(t_counts[c][d] for c in range(NCHUNK))
                               for d in dlist}
                    ps_agg = {d: psagp.tile([128, 128], F32, tag="psag",
                                            name=f"psag_{s}_{d}")
                              for d in dlist if total_d[d] > 0}
                    done_d = {d: 0 for d in dlist}
                    for c in range(NCHUNK):
                        seg = seg_tiles[s][c]
                        if seg == 0:
                            continue
                        gt = gbufp.tile([128, max_seg, 128], F32, tag="gbuf")
                        nc.gpsimd.dma_gather(
                            gt[:, :seg, :],
                            table[c * CHUNK_ROWS:(c + 1) * CHUNK_ROWS, :],
                            gix_s[:, t0 * 8:(t0 + seg) * 8],
                            seg * 128, seg * 128, D,
                            single_packet=False)
                        tt = t0
                        for d in dlist:
                            for _ in range(t_counts[c][d]):
                                s_t = sbldp.tile([128, 128], F32, tag="sbld")
                                nc.vector.tensor_scalar(
                                    s_t[:], iota_s[:],
                                    colw_s[:, tt:tt + 1], nrmw_s[:, tt:tt + 1],
                                    mybir.AluOpType.is_equal,
                                    mybir.AluOpType.mult)
                                nc.tensor.matmul(
                                    ps_agg[d][:], s_t[:], gt[:, tt - t0, :],
                                    start=(done_d[d] == 0),
                                    stop=(done_d[d] == total_d[d] - 1))
                                done_d[d] += 1
                                tt += 1
                        t0 += seg
                    for d in dlist:
                        xsel = tailp.tile([128, 128], F32, tag="xsel")
                        nc.sync.dma_start(
                            xsel[:], self_src[d * 128:(d + 1) * 128, :])
                        agg_sb = tailp.tile([128, 128], F32, tag="aggsb")
                        if total_d[d] > 0:
                            nc.vector.scalar_tensor_tensor(
                                agg_sb[:], xsel[:], selfw_s[:, d:d + 1],
                                ps_agg[d][:],
                                mybir.AluOpType.mult, mybir.AluOpType.add)
                        else:
                            nc.vector.tensor_scalar(
                                agg_sb[:], xsel[:], selfw_s[:, d:d + 1], None,
                                mybir.AluOpType.mult)
                        ps_t = pstp.tile([128, 128], F32, tag="pst")
                        nc.tensor.transpose(ps_t[:], agg_sb[:], id_s[:])
                        agg_t = tailp.tile([128, 128], F32, tag="aggt")
                        nc.scalar.activation(
                            agg_t[:], ps_t[:],
                            mybir.ActivationFunctionType.Copy)
                        ps_o = psop.tile([128, 128], F32, tag="pso")
                        nc.tensor.matmul(ps_o[:], agg_t[:], w_s[:],
                                         start=True, stop=False)
                        nc.tensor.matmul(ps_o[:], ones_s[:], b_s[:],
                                         start=False, stop=True)
                        o_sb = tailp.tile([128, 128], F32, tag="osb")
                        func = (mybir.ActivationFunctionType.Relu if relu
                                else mybir.ActivationFunctionType.Copy)
                        nc.scalar.activation(o_sb[:], ps_o[:], func)
                        store(d, o_sb)

            def store1(d, o_sb):
                nc.sync.dma_start(h_local[d * 128:(d + 1) * 128, :], o_sb[:])

            def store2(d, o_sb):
                lo = d * 128
                hi = min(lo + 128, NPC)
                nc.sync.dma_start(out_slab[lo:hi, :], o_sb[:hi - lo, :])

            layer(x_full, x_slab, w1_s, b1_s, True, store1)
            nc.gpsimd.collective_compute(
                "AllGather", mybir.AluOpType.bypass,
                replica_groups=[list(range(N_CORES))],
                ins=[h_local.opt()], outs=[h_full.opt()])
            layer(h_full, h_local, w2_s, b2_s, False, store2)

    nc.compile()
    return nc


def _preprocess(x, edge_index, edge_weight):
    """Host-side graph preprocessing -> uniform structure + per-core inputs."""
    row = np.asarray(edge_index[0], dtype=np.int64)
    col = np.asarray(edge_index[1], dtype=np.int64)
    ew = np.asarray(edge_weight, dtype=np.float32)
    n_nodes = N_NODES

    deg = np.bincount(col, weights=ew.astype(np.float64), minlength=n_nodes)
    deg = (deg + 1.0).astype(np.float32)
    dinv = (1.0 / np.sqrt(deg)).astype(np.float32)
    nrm = (dinv[row] * ew * dinv[col]).astype(np.float32)
    selfw_n = (dinv * dinv).astype(np.float32)

    core = col // NPC
    dtile = (col - core * NPC) >> 7
    src_core = row // NPC
    pad_row = (src_core * PAD_NPC + (row - src_core * NPC)).astype(np.int64)
    chunk = pad_row // CHUNK_ROWS
    sup = dtile // SUPER

    # order: (core, super, chunk, dtile)
    key = ((core * ((TPC + SUPER - 1) // SUPER) + sup) * NCHUNK + chunk) * TPC + dtile
    order = np.argsort(key, kind="stable")
    kcd = (core * NCHUNK + chunk) * TPC + dtile   # per (core,chunk,dtile) counts
    counts = np.bincount(kcd, minlength=N_CORES * NCHUNK * TPC)
    counts = counts.reshape(N_CORES, NCHUNK, TPC)
    t_counts = -(-counts.max(axis=0) // 128)       # [NCHUNK, TPC], may be 0

    # slot base per (chunk, dtile) in (super, chunk, dtile) stream order
    NSUP = (TPC + SUPER - 1) // SUPER
    slot_base = np.zeros((NCHUNK, TPC), np.int64)
    acc = 0
    for s in range(NSUP):
        for c in range(NCHUNK):
            for d in range(s * SUPER, min((s + 1) * SUPER, TPC)):
                slot_base[c, d] = acc
                acc += int(t_counts[c, d])
    NT = int(acc)

    key_s = key[order]
    group_start = np.concatenate(
        [[0], np.cumsum(np.bincount(key_s, minlength=key.max() + 1))[:-1]])
    rank = np.arange(len(key_s)) - group_start[key_s]

    gix = np.zeros((N_CORES, NT * 128), np.int16)
    colw = np.zeros((N_CORES, NT * 128), np.float32)
    nrmw = np.zeros((N_CORES, NT * 128), np.float32)
    pos = slot_base[chunk[order], dtile[order]] * 128 + rank
    cidx = core[order]
    gix[cidx, pos] = (pad_row[order] - chunk[order] * CHUNK_ROWS).astype(np.int16)
    colw[cidx, pos] = ((col - core * NPC)[order] & 127).astype(np.float32)
    nrmw[cidx, pos] = nrm[order]

    # wrap gather indices: idx j at partition (16g + j%16), slot j//16
    gixw = gix.reshape(N_CORES, NT * 8, 16).transpose(0, 2, 1)   # [C,16,NT*8]
    gixw = np.ascontiguousarray(np.tile(gixw, (1, 8, 1)))        # [C,128,NT*8]
    colw = np.ascontiguousarray(
        colw.reshape(N_CORES, NT, 128).transpose(0, 2, 1))
    nrmw = np.ascontiguousarray(
        nrmw.reshape(N_CORES, NT, 128).transpose(0, 2, 1))

    selfw_pad = np.zeros(N_CORES * PAD_NPC, np.float32)
    idx_all = np.arange(n_nodes)
    c_all = idx_all // NPC
    selfw_pad[c_all * PAD_NPC + (idx_all - c_all * NPC)] = selfw_n
    selfw = np.ascontiguousarray(
        selfw_pad.reshape(N_CORES, TPC, 128).transpose(0, 2, 1))

    x = np.asarray(x, dtype=np.float32)
    x_slabs = np.zeros((N_CORES, PAD_NPC, D), np.float32)
    x_slabs[:, :NPC, :] = x.reshape(N_CORES, NPC, D)

    t_key = tuple(tuple(int(v) for v in t_counts[c]) for c in range(NCHUNK))
    return t_key, gixw, colw, nrmw, selfw, x_slabs


_NC_CACHE: dict = {}


def kernel(x, edge_index, edge_weight, W1, b1, W2, b2):
    t_key, gixw, colw, nrmw, selfw, x_slabs = _preprocess(
        x, edge_index, edge_weight)

    if t_key not in _NC_CACHE:
        _NC_CACHE[t_key] = _build_nc([list(r) for r in t_key])
    nc = _NC_CACHE[t_key]

    w1_np = np.ascontiguousarray(np.asarray(W1, dtype=np.float32))
    w2_np = np.ascontiguousarray(np.asarray(W2, dtype=np.float32))
    b1_np = np.asarray(b1, dtype=np.float32).reshape(1, D)
    b2_np = np.asarray(b2, dtype=np.float32).reshape(1, D)
    ident = np.eye(128, dtype=np.float32)
    iota = np.tile(np.arange(128, dtype=np.float32), (128, 1))
    ones = np.ones((1, D), np.float32)

    in_maps = []
    for c in range(N_CORES):
        in_maps.append({
            "x_slab": x_slabs[c], "gix": gixw[c], "colw": colw[c],
            "nrmw": nrmw[c], "selfw": selfw[c],
            "w1": w1_np, "b1": b1_np, "w2": w2_np, "b2": b2_np,
            "ident": ident, "iota": iota, "ones": ones,
        })

    res = run_bass_kernel_spmd(nc, in_maps, core_ids=list(range(N_CORES)))
    out = np.concatenate([res.results[c]["out_slab"] for c in range(N_CORES)],
                         axis=0)
    return out


# revision 7
# speedup vs baseline: 1.7634x; 1.7634x over previous
"""Trainium2 Bass kernel for a 2-layer directed GCN (PyG GCNConv semantics).

Strategy (8-core SPMD, 1D node sharding):
  - Nodes sharded across 8 cores (12500 each, padded to 12544 = 98*128).
  - Edges partitioned by destination core; per core grouped by destination
    tile (128 nodes), sub-grouped by source chunk (int16 gather reach), and
    padded to whole 128-edge tiles with a cross-core-uniform tile count so a
    single SPMD program serves all cores.
  - Per-edge coefficients nrm = dinv[row]*ew*dinv[col] and self-loop weights
    dinv^2 are host-precomputed (graph-structure preprocessing, cacheable).
  - Linearity trick: aggregate raw features first, apply W afterwards:
        out[c] = (sum_e nrm_e * x[row_e] + dinv_c^2 * x_c) @ W + b
    so layer 1 gathers raw x (replicated bf16 table input, no collective);
    one AllGather of the bf16 relu output builds the layer-2 table.
  - Per 128-edge tile on device: batched gpsimd dma_gather (bf16 rows),
    scaled one-hot S[e,d] = (iota==col)*nrm via one fused DVE tensor_scalar
    (bf16, 2x mode), TensorE bf16 matmul S^T @ msgs accumulating the
    destination tile in fp32 PSUM. Supertiles of 6 destination tiles keep 6
    PSUM aggregation banks live across the 4 source-chunk passes
    (+1 transpose +1 output bank = 8).
  - Per destination tile tail (fp32): self term (DVE scalar_tensor_tensor),
    PE transpose, x W (PE), bias via K=1 rank-1 matmul, relu/copy evict on
    the scalar engine (plus a bf16 relu evict feeding the layer-2 table).
"""

from contextlib import ExitStack

import ml_dtypes
import numpy as np

import concourse.bacc as bacc
import concourse.bass as bass
import concourse.mybir as mybir
import concourse.tile as tile
from concourse.bass_utils import run_bass_kernel_spmd
from concourse.library_config import mlp

N_NODES = 100000
D = 128
N_CORES = 8
NPC = N_NODES // N_CORES          # 12500 nodes per core
TPC = (NPC + 127) // 128          # 98 destination tiles per core
PAD_NPC = TPC * 128               # 12544 padded nodes per core
N_PAD = N_CORES * PAD_NPC         # 100352 padded table rows
NCHUNK = 4
CHUNK_ROWS = N_PAD // NCHUNK      # 25088 (< 32768, fits int16 indices)
SUPER = 6                         # dest tiles per supertile (PSUM: 6+1+1)

F32 = mybir.dt.float32
BF16 = mybir.dt.bfloat16
I16 = mybir.dt.int16
NPBF = ml_dtypes.bfloat16


def _build_nc(t_counts):
    """t_counts[chunk][dtile] = edge tiles (uniform across cores)."""
    NSUP = (TPC + SUPER - 1) // SUPER
    NT = int(sum(t_counts[c][d] for c in range(NCHUNK) for d in range(TPC)))
    nc = bacc.Bacc("TRN2", target_bir_lowering=False)

    x_slab = nc.dram_tensor("x_slab", [PAD_NPC, D], F32, kind="ExternalInput")
    x_tab = nc.dram_tensor("x_tab", [N_PAD, D], BF16, kind="ExternalInput")
    gix = nc.dram_tensor("gix", [128, NT * 8], I16, kind="ExternalInput")
    colw = nc.dram_tensor("colw", [128, NT], F32, kind="ExternalInput")
    nrmw = nc.dram_tensor("nrmw", [128, NT], F32, kind="ExternalInput")
    selfw = nc.dram_tensor("selfw", [128, TPC], F32, kind="ExternalInput")
    w1 = nc.dram_tensor("w1", [D, D], F32, kind="ExternalInput")
    b1 = nc.dram_tensor("b1", [1, D], F32, kind="ExternalInput")
    w2 = nc.dram_tensor("w2", [D, D], F32, kind="ExternalInput")
    b2 = nc.dram_tensor("b2", [1, D], F32, kind="ExternalInput")
    ident = nc.dram_tensor("ident", [128, 128], F32, kind="ExternalInput")
    iota = nc.dram_tensor("iota", [128, 128], BF16, kind="ExternalInput")
    ones = nc.dram_tensor("ones", [1, D], F32, kind="ExternalInput")
    out_slab = nc.dram_tensor("out_slab", [NPC, D], F32, kind="ExternalOutput")

    seg_tiles = [[sum(t_counts[c][d]
                      for d in range(s * SUPER, min((s + 1) * SUPER, TPC)))
                  for c in range(NCHUNK)] for s in range(NSUP)]
    max_seg = max(max(row) for row in seg_tiles) or 1

    with tile.TileContext(nc) as tc:
        nc.gpsimd.load_library(mlp)
        with (
            tc.tile_pool(name="const", bufs=1) as constp,
            tc.tile_pool(name="gbuf", bufs=3) as gbufp,
            tc.tile_pool(name="sbld", bufs=4) as sbldp,
            tc.tile_pool(name="tailp", bufs=3) as tailp,
            tc.tile_pool(name="psag", bufs=SUPER, space="PSUM") as psagp,
            tc.tile_pool(name="pst", bufs=1, space="PSUM") as pstp,
            tc.tile_pool(name="pso", bufs=1, space="PSUM") as psop,
            tc.tile_pool(name="dram", bufs=1, space="DRAM") as dramp,
        ):
            gix_s = constp.tile([128, NT * 8], I16, tag="gix")
            colw_s = constp.tile([128, NT], F32, tag="colw")
            nrmw_s = constp.tile([128, NT], F32, tag="nrmw")
            selfw_s = constp.tile([128, TPC], F32, tag="selfw")
            w1_s = constp.tile([D, D], F32, tag="w1")
            b1_s = constp.tile([1, D], F32, tag="b1")
            w2_s = constp.tile([D, D], F32, tag="w2")
            b2_s = constp.tile([1, D], F32, tag="b2")
            id_s = constp.tile([128, 128], F32, tag="ident")
            iota_s = constp.tile([128, 128], BF16, tag="iota")
            ones_s = constp.tile([1, D], F32, tag="ones")
            for dst, srct in ((gix_s, gix), (colw_s, colw), (nrmw_s, nrmw),
                              (selfw_s, selfw), (w1_s, w1), (b1_s, b1),
                              (w2_s, w2), (b2_s, b2), (id_s, ident),
                              (iota_s, iota), (ones_s, ones)):
                nc.sync.dma_start(dst[:], srct[:])

            h_local = dramp.tile([PAD_NPC, D], F32, tag="h_local")
            h_loc_bf = dramp.tile([PAD_NPC, D], BF16, tag="h_loc_bf")
            h_full = dramp.tile([N_PAD, D], BF16, tag="h_full",
                                addr_space="Shared")

            def layer(table, self_src, w_s, b_s, relu, store):
                t0 = 0
                for s in range(NSUP):
                    dlist = list(range(s * SUPER, min((s + 1) * SUPER, TPC)))
                    total_d = {d: sum(t_counts[c][d] for c in range(NCHUNK))
                               for d in dlist}
                    ps_agg = {d: psagp.tile([128, 128], F32, tag="psag",
                                            name=f"psag_{s}_{d}")
                              for d in dlist if total_d[d] > 0}
                    done_d = {d: 0 for d in dlist}
                    for c in range(NCHUNK):
                        seg = seg_tiles[s][c]
                        if seg == 0:
                            continue
                        gt = gbufp.tile([128, max_seg, 128], BF16, tag="gbuf")
                        nc.gpsimd.dma_gather(
                            gt[:, :seg, :],
                            table[c * CHUNK_ROWS:(c + 1) * CHUNK_ROWS, :],
                            gix_s[:, t0 * 8:(t0 + seg) * 8],
                            seg * 128, seg * 128, D,
                            single_packet=False)
                        tt = t0
                        for d in dlist:
                            for _ in range(t_counts[c][d]):
                                s_t = sbldp.tile([128, 128], BF16, tag="sbld")
                                nc.vector.tensor_scalar(
                                    s_t[:], iota_s[:],
                                    colw_s[:, tt:tt + 1], nrmw_s[:, tt:tt + 1],
                                    mybir.AluOpType.is_equal,
                                    mybir.AluOpType.mult)
                                nc.tensor.matmul(
                                    ps_agg[d][:], s_t[:], gt[:, tt - t0, :],
                                    start=(done_d[d] == 0),
                                    stop=(done_d[d] == total_d[d] - 1))
                                done_d[d] += 1
                                tt += 1
                        t0 += seg
                    for d in dlist:
                        xsel = tailp.tile([128, 128], F32, tag="xsel")
                        nc.sync.dma_start(
                            xsel[:], self_src[d * 128:(d + 1) * 128, :])
                        agg_sb = tailp.tile([128, 128], F32, tag="aggsb")
                        if total_d[d] > 0:
                            nc.vector.scalar_tensor_tensor(
                                agg_sb[:], xsel[:], selfw_s[:, d:d + 1],
                                ps_agg[d][:],
                                mybir.AluOpType.mult, mybir.AluOpType.add)
                        else:
                            nc.vector.tensor_scalar(
                                agg_sb[:], xsel[:], selfw_s[:, d:d + 1], None,
                                mybir.AluOpType.mult)
                        ps_t = pstp.tile([128, 128], F32, tag="pst")
                        nc.tensor.transpose(ps_t[:], agg_sb[:], id_s[:])
                        agg_t = tailp.tile([128, 128], F32, tag="aggt")
                        nc.scalar.activation(
                            agg_t[:], ps_t[:],
                            mybir.ActivationFunctionType.Copy)
                        ps_o = psop.tile([128, 128], F32, tag="pso")
                        nc.tensor.matmul(ps_o[:], agg_t[:], w_s[:],
                                         start=True, stop=False)
                        nc.tensor.matmul(ps_o[:], ones_s[:], b_s[:],
                                         start=False, stop=True)
                        o_sb = tailp.tile([128, 128], F32, tag="osb")
                        func = (mybir.ActivationFunctionType.Relu if relu
                                else mybir.ActivationFunctionType.Copy)
                        nc.scalar.activation(o_sb[:], ps_o[:], func)
                        store(d, ps_o, o_sb)

            def store1(d, ps_o, o_sb):
                nc.sync.dma_start(h_local[d * 128:(d + 1) * 128, :], o_sb[:])
                hbf = tailp.tile([128, 128], BF16, tag="hbf")
                nc.scalar.activation(hbf[:], ps_o[:],
                                     mybir.ActivationFunctionType.Relu)
                nc.sync.dma_start(h_loc_bf[d * 128:(d + 1) * 128, :], hbf[:])

            def store2(d, ps_o, o_sb):
                lo = d * 128
                hi = min(lo + 128, NPC)
                nc.sync.dma_start(out_slab[lo:hi, :], o_sb[:hi - lo, :])

            layer(x_tab, x_slab, w1_s, b1_s, True, store1)
            nc.gpsimd.collective_compute(
                "AllGather", mybir.AluOpType.bypass,
                replica_groups=[list(range(N_CORES))],
                ins=[h_loc_bf.opt()], outs=[h_full.opt()])
            layer(h_full, h_local, w2_s, b2_s, False, store2)

    nc.compile()
    return nc


def _preprocess(x, edge_index, edge_weight):
    """Host-side graph preprocessing -> uniform structure + per-core inputs."""
    row = np.asarray(edge_index[0], dtype=np.int64)
    col = np.asarray(edge_index[1], dtype=np.int64)
    ew = np.asarray(edge_weight, dtype=np.float32)
    n_nodes = N_NODES

    deg = np.bincount(col, weights=ew.astype(np.float64), minlength=n_nodes)
    deg = (deg + 1.0).astype(np.float32)
    dinv = (1.0 / np.sqrt(deg)).astype(np.float32)
    nrm = (dinv[row] * ew * dinv[col]).astype(np.float32)
    selfw_n = (dinv * dinv).astype(np.float32)

    core = col // NPC
    dtile = (col - core * NPC) >> 7
    src_core = row // NPC
    pad_row = (src_core * PAD_NPC + (row - src_core * NPC)).astype(np.int64)
    chunk = pad_row // CHUNK_ROWS
    sup = dtile // SUPER

    key = ((core * ((TPC + SUPER - 1) // SUPER) + sup) * NCHUNK + chunk) * TPC + dtile
    order = np.argsort(key, kind="stable")
    kcd = (core * NCHUNK + chunk) * TPC + dtile
    counts = np.bincount(kcd, minlength=N_CORES * NCHUNK * TPC)
    counts = counts.reshape(N_CORES, NCHUNK, TPC)
    t_counts = -(-counts.max(axis=0) // 128)       # [NCHUNK, TPC], may be 0

    NSUP = (TPC + SUPER - 1) // SUPER
    slot_base = np.zeros((NCHUNK, TPC), np.int64)
    acc = 0
    for s in range(NSUP):
        for c in range(NCHUNK):
            for d in range(s * SUPER, min((s + 1) * SUPER, TPC)):
                slot_base[c, d] = acc
                acc += int(t_counts[c, d])
    NT = int(acc)

    key_s = key[order]
    group_start = np.concatenate(
        [[0], np.cumsum(np.bincount(key_s, minlength=key.max() + 1))[:-1]])
    rank = np.arange(len(key_s)) - group_start[key_s]

    gixf = np.zeros((N_CORES, NT * 128), np.int16)
    colwf = np.zeros((N_CORES, NT * 128), np.float32)
    nrmwf = np.zeros((N_CORES, NT * 128), np.float32)
    pos = slot_base[chunk[order], dtile[order]] * 128 + rank
    cidx = core[order]
    gixf[cidx, pos] = (pad_row[order] - chunk[order] * CHUNK_ROWS).astype(np.int16)
    colwf[cidx, pos] = ((col - core * NPC)[order] & 127).astype(np.float32)
    nrmwf[cidx, pos] = nrm[order]

    gixw = gixf.reshape(N_CORES, NT * 8, 16).transpose(0, 2, 1)
    gixw = np.ascontiguousarray(np.tile(gixw, (1, 8, 1)))        # [C,128,NT*8]
    colw = np.ascontiguousarray(
        colwf.reshape(N_CORES, NT, 128).transpose(0, 2, 1))
    nrmw = np.ascontiguousarray(
        nrmwf.reshape(N_CORES, NT, 128).transpose(0, 2, 1))

    selfw_pad = np.zeros(N_CORES * PAD_NPC, np.float32)
    idx_all = np.arange(n_nodes)
    c_all = idx_all // NPC
    selfw_pad[c_all * PAD_NPC + (idx_all - c_all * NPC)] = selfw_n
    selfw = np.ascontiguousarray(
        selfw_pad.reshape(N_CORES, TPC, 128).transpose(0, 2, 1))

    x = np.asarray(x, dtype=np.float32)
    x_slabs = np.zeros((N_CORES, PAD_NPC, D), np.float32)
    x_slabs[:, :NPC, :] = x.reshape(N_CORES, NPC, D)
    x_tab = np.zeros((N_PAD, D), NPBF)
    x_tab.reshape(N_CORES, PAD_NPC, D)[:, :NPC, :] = \
        x.reshape(N_CORES, NPC, D).astype(NPBF)

    t_key = tuple(tuple(int(v) for v in t_counts[c]) for c in range(NCHUNK))
    return t_key, gixw, colw, nrmw, selfw, x_slabs, x_tab


_NC_CACHE: dict = {}


def kernel(x, edge_index, edge_weight, W1, b1, W2, b2):
    t_key, gixw, colw, nrmw, selfw, x_slabs, x_tab = _preprocess(
        x, edge_index, edge_weight)

    if t_key not in _NC_CACHE:
        _NC_CACHE[t_key] = _build_nc([list(r) for r in t_key])
    nc = _NC_CACHE[t_key]

    w1_np = np.ascontiguousarray(np.asarray(W1, dtype=np.float32))
    w2_np = np.ascontiguousarray(np.asarray(W2, dtype=np.float32))
    b1_np = np.asarray(b1, dtype=np.float32).reshape(1, D)
    b2_np = np.asarray(b2, dtype=np.float32).reshape(1, D)
    ident = np.eye(128, dtype=np.float32)
    iota = np.tile(np.arange(128), (128, 1)).astype(NPBF)
    ones = np.ones((1, D), np.float32)

    in_maps = []
    for c in range(N_CORES):
        in_maps.append({
            "x_slab": x_slabs[c], "x_tab": x_tab, "gix": gixw[c],
            "colw": colw[c], "nrmw": nrmw[c], "selfw": selfw[c],
            "w1": w1_np, "b1": b1_np, "w2": w2_np, "b2": b2_np,
            "ident": ident, "iota": iota, "ones": ones,
        })

    res = run_bass_kernel_spmd(nc, in_maps, core_ids=list(range(N_CORES)))
    out = np.concatenate([res.results[c]["out_slab"] for c in range(N_CORES)],
                         axis=0)
    return out


# revision 13
# speedup vs baseline: 1.9032x; 1.0793x over previous
"""Trainium2 Bass kernel for a 2-layer directed GCN (PyG GCNConv semantics).

Strategy (8-core SPMD, 1D node sharding):
  - Nodes sharded across 8 cores (12500 each, padded to 12544 = 98*128).
  - Edges partitioned by destination core; per core grouped by destination
    tile (128 nodes), sub-grouped by source chunk (int16 gather reach), and
    padded to whole 128-edge tiles with a cross-core-uniform tile count so a
    single SPMD program serves all cores.
  - Per-edge coefficients nrm = dinv[row]*ew*dinv[col] and self-loop weights
    dinv^2 are host-precomputed (graph-structure preprocessing, cacheable).
  - Linearity trick: aggregate raw features first, apply W afterwards:
        out[c] = (sum_e nrm_e * x[row_e] + dinv_c^2 * x_c) @ W + b
    so layer 1 gathers raw x (replicated bf16 table input, no collective);
    one AllGather of the bf16 relu output builds the layer-2 table.
  - Per 128-edge tile on device: batched gpsimd dma_gather (bf16 rows),
    scaled one-hot S[e,d] = (iota==col)*nrm via one fused DVE tensor_scalar
    (bf16, 2x mode), TensorE bf16 matmul S^T @ msgs accumulating the
    destination tile in fp32 PSUM. Supertiles of 4 destination tiles keep 4
    PSUM aggregation banks live across the 4 source-chunk passes
    (+2 transpose +2 output banks = 8).
  - Per destination tile tail (fp32): self term (DVE scalar_tensor_tensor),
    PE transpose, x W (PE), bias via K=1 rank-1 matmul, relu/copy evict on
    the scalar engine. Layer-1 output is stored once, as bf16: it is both
    the AllGather payload for the layer-2 gather table and the layer-2
    self-term source.
"""

import ml_dtypes
import numpy as np

import concourse.bacc as bacc
import concourse.mybir as mybir
import concourse.tile as tile
from concourse.bass_utils import run_bass_kernel_spmd
from concourse.library_config import mlp

N_NODES = 100000
D = 128
N_CORES = 8
NPC = N_NODES // N_CORES          # 12500 nodes per core
TPC = (NPC + 127) // 128          # 98 destination tiles per core
PAD_NPC = TPC * 128               # 12544 padded nodes per core
N_PAD = N_CORES * PAD_NPC         # 100352 padded table rows
NCHUNK = 4
CHUNK_ROWS = N_PAD // NCHUNK      # 25088 (< 32768, fits int16 indices)
SUPER = 4                         # dest tiles per supertile (PSUM: 4+2+2)

F32 = mybir.dt.float32
BF16 = mybir.dt.bfloat16
I16 = mybir.dt.int16
NPBF = ml_dtypes.bfloat16


def _build_nc(t_counts):
    """t_counts[chunk][dtile] = edge tiles (uniform across cores)."""
    NSUP = (TPC + SUPER - 1) // SUPER
    NT = int(sum(t_counts[c][d] for c in range(NCHUNK) for d in range(TPC)))
    nc = bacc.Bacc("TRN2", target_bir_lowering=False)

    x_slab = nc.dram_tensor("x_slab", [PAD_NPC, D], F32, kind="ExternalInput")
    x_tab = nc.dram_tensor("x_tab", [N_PAD, D], BF16, kind="ExternalInput")
    gix = nc.dram_tensor("gix", [128, NT * 8], I16, kind="ExternalInput")
    colw = nc.dram_tensor("colw", [128, NT], F32, kind="ExternalInput")
    nrmw = nc.dram_tensor("nrmw", [128, NT], F32, kind="ExternalInput")
    selfw = nc.dram_tensor("selfw", [128, TPC], F32, kind="ExternalInput")
    w1 = nc.dram_tensor("w1", [D, D], F32, kind="ExternalInput")
    b1 = nc.dram_tensor("b1", [1, D], F32, kind="ExternalInput")
    w2 = nc.dram_tensor("w2", [D, D], F32, kind="ExternalInput")
    b2 = nc.dram_tensor("b2", [1, D], F32, kind="ExternalInput")
    ident = nc.dram_tensor("ident", [128, 128], F32, kind="ExternalInput")
    iota = nc.dram_tensor("iota", [128, 128], BF16, kind="ExternalInput")
    ones = nc.dram_tensor("ones", [1, D], F32, kind="ExternalInput")
    out_slab = nc.dram_tensor("out_slab", [NPC, D], F32, kind="ExternalOutput")

    seg_tiles = [[sum(t_counts[c][d]
                      for d in range(s * SUPER, min((s + 1) * SUPER, TPC)))
                  for c in range(NCHUNK)] for s in range(NSUP)]
    max_seg = max(max(row) for row in seg_tiles) or 1

    with tile.TileContext(nc) as tc:
        nc.gpsimd.load_library(mlp)
        with (
            tc.tile_pool(name="const", bufs=1) as constp,
            tc.tile_pool(name="gbuf", bufs=4) as gbufp,
            tc.tile_pool(name="sbld", bufs=8) as sbldp,
            tc.tile_pool(name="tailp", bufs=6) as tailp,
            tc.tile_pool(name="psag", bufs=SUPER, space="PSUM") as psagp,
            tc.tile_pool(name="pst", bufs=2, space="PSUM") as pstp,
            tc.tile_pool(name="pso", bufs=2, space="PSUM") as psop,
            tc.tile_pool(name="dram", bufs=1, space="DRAM") as dramp,
        ):
            gix_s = constp.tile([128, NT * 8], I16, tag="gix")
            colw_s = constp.tile([128, NT], F32, tag="colw")
            nrmw_s = constp.tile([128, NT], F32, tag="nrmw")
            selfw_s = constp.tile([128, TPC], F32, tag="selfw")
            w1_s = constp.tile([D, D], F32, tag="w1")
            b1_s = constp.tile([1, D], F32, tag="b1")
            w2_s = constp.tile([D, D], F32, tag="w2")
            b2_s = constp.tile([1, D], F32, tag="b2")
            id_s = constp.tile([128, 128], F32, tag="ident")
            iota_s = constp.tile([128, 128], BF16, tag="iota")
            ones_s = constp.tile([1, D], F32, tag="ones")
            for dst, srct in ((gix_s, gix), (colw_s, colw), (nrmw_s, nrmw),
                              (selfw_s, selfw), (w1_s, w1), (b1_s, b1),
                              (w2_s, w2), (b2_s, b2), (id_s, ident),
                              (iota_s, iota), (ones_s, ones)):
                nc.sync.dma_start(dst[:], srct[:])

            h_loc_bf = dramp.tile([PAD_NPC, D], BF16, tag="h_loc_bf")
            h_full = dramp.tile([N_PAD, D], BF16, tag="h_full",
                                addr_space="Shared")

            def layer(table, self_src, w_s, b_s, relu, store):
                t0 = 0
                for s in range(NSUP):
                    dlist = list(range(s * SUPER, min((s + 1) * SUPER, TPC)))
                    total_d = {d: sum(t_counts[c][d] for c in range(NCHUNK))
                               for d in dlist}
                    ps_agg = {d: psagp.tile([128, 128], F32, tag="psag",
                                            name=f"psag_{s}_{d}")
                              for d in dlist if total_d[d] > 0}
                    done_d = {d: 0 for d in dlist}
                    for c in range(NCHUNK):
                        seg = seg_tiles[s][c]
                        if seg == 0:
                            continue
                        gt = gbufp.tile([128, max_seg, 128], BF16, tag="gbuf")
                        nc.gpsimd.dma_gather(
                            gt[:, :seg, :],
                            table[c * CHUNK_ROWS:(c + 1) * CHUNK_ROWS, :],
                            gix_s[:, t0 * 8:(t0 + seg) * 8],
                            seg * 128, seg * 128, D,
                            single_packet=False)
                        tt = t0
                        for d in dlist:
                            for _ in range(t_counts[c][d]):
                                s_t = sbldp.tile([128, 128], BF16, tag="sbld")
                                nc.vector.tensor_scalar(
                                    s_t[:], iota_s[:],
                                    colw_s[:, tt:tt + 1], nrmw_s[:, tt:tt + 1],
                                    mybir.AluOpType.is_equal,
                                    mybir.AluOpType.mult)
                                nc.tensor.matmul(
                                    ps_agg[d][:], s_t[:], gt[:, tt - t0, :],
                                    start=(done_d[d] == 0),
                                    stop=(done_d[d] == total_d[d] - 1))
                                done_d[d] += 1
                                tt += 1
                        t0 += seg
                    for d in dlist:
                        xsel = tailp.tile([128, 128],
                                          F32 if self_src is x_slab else BF16,
                                          tag="xsel")
                        nc.sync.dma_start(
                            xsel[:], self_src[d * 128:(d + 1) * 128, :])
                        agg_sb = tailp.tile([128, 128], F32, tag="aggsb")
                        if total_d[d] > 0:
                            nc.vector.scalar_tensor_tensor(
                                agg_sb[:], xsel[:], selfw_s[:, d:d + 1],
                                ps_agg[d][:],
                                mybir.AluOpType.mult, mybir.AluOpType.add)
                        else:
                            nc.vector.tensor_scalar(
                                agg_sb[:], xsel[:], selfw_s[:, d:d + 1], None,
                                mybir.AluOpType.mult)
                        ps_t = pstp.tile([128, 128], F32, tag="pst")
                        nc.tensor.transpose(ps_t[:], agg_sb[:], id_s[:])
                        agg_t = tailp.tile([128, 128], F32, tag="aggt")
                        nc.scalar.activation(
                            agg_t[:], ps_t[:],
                            mybir.ActivationFunctionType.Copy)
                        ps_o = psop.tile([128, 128], F32, tag="pso")
                        nc.tensor.matmul(ps_o[:], agg_t[:], w_s[:],
                                         start=True, stop=False)
                        nc.tensor.matmul(ps_o[:], ones_s[:], b_s[:],
                                         start=False, stop=True)
                        o_sb = tailp.tile([128, 128],
                                          BF16 if relu else F32, tag="osb")
                        func = (mybir.ActivationFunctionType.Relu if relu
                                else mybir.ActivationFunctionType.Copy)
                        nc.scalar.activation(o_sb[:], ps_o[:], func)
                        store(d, ps_o, o_sb)

            def store1(d, ps_o, o_sb):
                nc.sync.dma_start(h_loc_bf[d * 128:(d + 1) * 128, :], o_sb[:])

            def store2(d, ps_o, o_sb):
                lo = d * 128
                hi = min(lo + 128, NPC)
                nc.sync.dma_start(out_slab[lo:hi, :], o_sb[:hi - lo, :])

            layer(x_tab, x_slab, w1_s, b1_s, True, store1)
            nc.gpsimd.collective_compute(
                "AllGather", mybir.AluOpType.bypass,
                replica_groups=[list(range(N_CORES))],
                ins=[h_loc_bf.opt()], outs=[h_full.opt()])
            layer(h_full, h_loc_bf, w2_s, b2_s, False, store2)

    nc.compile()
    return nc


def _preprocess(x, edge_index, edge_weight):
    """Host-side graph preprocessing -> uniform structure + per-core inputs."""
    row = np.asarray(edge_index[0], dtype=np.int64)
    col = np.asarray(edge_index[1], dtype=np.int64)
    ew = np.asarray(edge_weight, dtype=np.float32)
    n_nodes = N_NODES

    deg = np.bincount(col, weights=ew.astype(np.float64), minlength=n_nodes)
    deg = (deg + 1.0).astype(np.float32)
    dinv = (1.0 / np.sqrt(deg)).astype(np.float32)
    nrm = (dinv[row] * ew * dinv[col]).astype(np.float32)
    selfw_n = (dinv * dinv).astype(np.float32)

    core = col // NPC
    dtile = (col - core * NPC) >> 7
    src_core = row // NPC
    pad_row = (src_core * PAD_NPC + (row - src_core * NPC)).astype(np.int64)
    chunk = pad_row // CHUNK_ROWS
    sup = dtile // SUPER

    key = ((core * ((TPC + SUPER - 1) // SUPER) + sup) * NCHUNK + chunk) * TPC + dtile
    order = np.argsort(key, kind="stable")
    kcd = (core * NCHUNK + chunk) * TPC + dtile
    counts = np.bincount(kcd, minlength=N_CORES * NCHUNK * TPC)
    counts = counts.reshape(N_CORES, NCHUNK, TPC)
    t_counts = -(-counts.max(axis=0) // 128)       # [NCHUNK, TPC], may be 0

    NSUP = (TPC + SUPER - 1) // SUPER
    slot_base = np.zeros((NCHUNK, TPC), np.int64)
    acc = 0
    for s in range(NSUP):
        for c in range(NCHUNK):
            for d in range(s * SUPER, min((s + 1) * SUPER, TPC)):
                slot_base[c, d] = acc
                acc += int(t_counts[c, d])
    NT = int(acc)

    key_s = key[order]
    group_start = np.concatenate(
        [[0], np.cumsum(np.bincount(key_s, minlength=key.max() + 1))[:-1]])
    rank = np.arange(len(key_s)) - group_start[key_s]

    gixf = np.zeros((N_CORES, NT * 128), np.int16)
    colwf = np.zeros((N_CORES, NT * 128), np.float32)
    nrmwf = np.zeros((N_CORES, NT * 128), np.float32)
    pos = slot_base[chunk[order], dtile[order]] * 128 + rank
    cidx = core[order]
    gixf[cidx, pos] = (pad_row[order] - chunk[order] * CHUNK_ROWS).astype(np.int16)
    colwf[cidx, pos] = ((col - core * NPC)[order] & 127).astype(np.float32)
    nrmwf[cidx, pos] = nrm[order]

    gixw = gixf.reshape(N_CORES, NT * 8, 16).transpose(0, 2, 1)
    gixw = np.ascontiguousarray(np.tile(gixw, (1, 8, 1)))        # [C,128,NT*8]
    colw = np.ascontiguousarray(
        colwf.reshape(N_CORES, NT, 128).transpose(0, 2, 1))
    nrmw = np.ascontiguousarray(
        nrmwf.reshape(N_CORES, NT, 128).transpose(0, 2, 1))

    selfw_pad = np.zeros(N_CORES * PAD_NPC, np.float32)
    idx_all = np.arange(n_nodes)
    c_all = idx_all // NPC
    selfw_pad[c_all * PAD_NPC + (idx_all - c_all * NPC)] = selfw_n
    selfw = np.ascontiguousarray(
        selfw_pad.reshape(N_CORES, TPC, 128).transpose(0, 2, 1))

    x = np.asarray(x, dtype=np.float32)
    x_slabs = np.zeros((N_CORES, PAD_NPC, D), np.float32)
    x_slabs[:, :NPC, :] = x.reshape(N_CORES, NPC, D)
    x_tab = np.zeros((N_PAD, D), NPBF)
    x_tab.reshape(N_CORES, PAD_NPC, D)[:, :NPC, :] = \
        x.reshape(N_CORES, NPC, D).astype(NPBF)

    t_key = tuple(tuple(int(v) for v in t_counts[c]) for c in range(NCHUNK))
    return t_key, gixw, colw, nrmw, selfw, x_slabs, x_tab


_NC_CACHE: dict = {}


def kernel(x, edge_index, edge_weight, W1, b1, W2, b2):
    t_key, gixw, colw, nrmw, selfw, x_slabs, x_tab = _preprocess(
        x, edge_index, edge_weight)

    if t_key not in _NC_CACHE:
        _NC_CACHE[t_key] = _build_nc([list(r) for r in t_key])
    nc = _NC_CACHE[t_key]

    w1_np = np.ascontiguousarray(np.asarray(W1, dtype=np.float32))
    w2_np = np.ascontiguousarray(np.asarray(W2, dtype=np.float32))
    b1_np = np.asarray(b1, dtype=np.float32).reshape(1, D)
    b2_np = np.asarray(b2, dtype=np.float32).reshape(1, D)
    ident = np.eye(128, dtype=np.float32)
    iota = np.tile(np.arange(128), (128, 1)).astype(NPBF)
    ones = np.ones((1, D), np.float32)

    in_maps = []
    for c in range(N_CORES):
        in_maps.append({
            "x_slab": x_slabs[c], "x_tab": x_tab, "gix": gixw[c],
            "colw": colw[c], "nrmw": nrmw[c], "selfw": selfw[c],
            "w1": w1_np, "b1": b1_np, "w2": w2_np, "b2": b2_np,
            "ident": ident, "iota": iota, "ones": ones,
        })

    res = run_bass_kernel_spmd(nc, in_maps, core_ids=list(range(N_CORES)))
    out = np.concatenate([res.results[c]["out_slab"] for c in range(N_CORES)],
                         axis=0)
    return out
